# revision 1
# baseline (speedup 1.0000x reference)
"""BaseAttentivePool Trainium2 kernel (8-core SPMD).

Algorithm notes:
  - Segment softmax max-subtraction cancels mathematically:
      attn = exp(c - m)/sum(exp(c - m)) == exp(c)/sum(exp(c))
    so a single pass suffices: out = segsum(exp(c) * v) / (segsum(exp(c)) + eps).
  - Parents sharded 12500/core; children routed (host-side sort) to the core
    owning their parent, so all segment ops are core-local. No collectives.
  - Per core, parents are grouped in 98 windows of 128. Children of a window
    are padded to 128-multiples; a per-128-child-tile one-hot (built on-device
    by tensor_scalar is_equal against an iota row) feeds a PE matmul that
    scatter-accumulates [e*v | e] into the window PSUM accumulator.
  - Projections: child-major matmul, stationary = host-pretransposed
    [x | edge_attr | q_gather | 1] tile (fp16), moving = folded weight matrix.
    Host folds wkv/wk_rpe/wq_rpe/biases into one [106,128] fp16 matrix whose
    output columns are [k_eff(32) | v(64) | q_full(32)]; the q-gather rows of
    the stationary pass host-gathered parent queries through an identity
    block so q_full = q_gather + q_rpe + b_rpe comes out of the same matmul.
"""

import numpy as np

NC = 1_000_000
NP_ = 100_000
DIM = 64
H = 4
DQK = 8
DH = DQK * H
RPE = 9
SCALE = DQK ** -0.5

NCORES = 8
PPC = NP_ // NCORES            # 12500 parents per core
WIN = 128                      # parents per window
NWIN = -(-PPC // WIN)          # 98 windows (last has 84 parents)
CTILE = 128                    # children per tile
CHUNK = 8                      # tiles per DVE/ACT batch
LOAD_TILES = 16                # tiles per xq DMA

KROWS = DIM + RPE + DH + 1     # 106 live stationary rows: x|ea|qg|ones
XROWS = 128                    # padded to 128 partitions (DMA: [106,*]
                               # transfers run ~52GB/s vs ~330GB/s at 128)
F16 = np.float16

_BUILD_CACHE = {}


def _host_prep(x_child, x_parent, index, edge_attr, wq, bq, wkv, bkv,
               wk_rpe, bk_rpe, wq_rpe, bq_rpe):
    idx = np.asarray(index).astype(np.int64)
    x = np.asarray(x_child, dtype=np.float32)
    ea = np.asarray(edge_attr, dtype=np.float32)
    xp = np.asarray(x_parent, dtype=np.float32)

    # host: parent queries (tiny linear layer), gathered per child, fp16
    qp = xp @ (np.asarray(wq, np.float32) * SCALE) + np.asarray(bq, np.float32) * SCALE
    qg = qp[idx].astype(F16)

    core = idx // PPC
    lidx = idx - core * PPC
    w = lidx >> 7
    widx = (lidx & 127).astype(np.float32)

    order = np.argsort(idx, kind="stable")
    gid = (core * NWIN + w)[order]                      # sorted (core,window) id
    counts = np.bincount(gid, minlength=NCORES * NWIN).reshape(NCORES, NWIN)
    tw = -(-counts.max(axis=0) // CTILE)                # tiles per window (shared)
    tw = np.maximum(tw, 1)
    # pad total tiles to a LOAD_TILES multiple by growing the last window
    nt = int(tw.sum())
    pad_t = (-nt) % LOAD_TILES
    tw[-1] += pad_t
    nt += pad_t
    npc = nt * CTILE
    tile_off = np.concatenate([[0], np.cumsum(tw)])     # window -> first tile

    # destination slot of each sorted child within its core's padded layout
    seg_start = np.concatenate([[0], np.cumsum(counts.reshape(-1))])[:-1]
    rank = np.arange(NC) - seg_start[gid]
    dest = tile_off[w[order]] * CTILE + rank            # slot within core

    in_maps = []
    wcomb, bias = _fold_weights(wkv, bkv, wk_rpe, bk_rpe, wq_rpe, bq_rpe)
    iota = np.tile(np.arange(CTILE, dtype=F16), (CTILE, 1))
    core_sorted = core[order]
    for c in range(NCORES):
        sel = order[core_sorted == c]
        d = dest[core_sorted == c]
        A = np.zeros((npc, XROWS), np.float16)
        A[:, KROWS - 1] = 1.0
        A[d, 0:DIM] = x[sel].astype(F16)
        A[d, DIM:DIM + RPE] = ea[sel].astype(F16)
        A[d, DIM + RPE:DIM + RPE + DH] = qg[sel]
        xq = np.ascontiguousarray(A.T)                  # [106, npc] fp16
        wcol = np.full(npc, -1.0, np.float32)
        wcol[d] = widx[sel]
        widx_ct = np.ascontiguousarray(
            wcol.reshape(nt, CTILE).T.astype(np.float32))  # [128, nt]
        in_maps.append({"xq": xq, "widx": widx_ct, "wcomb": wcomb,
                        "iota": iota})
    return in_maps, tuple(int(t) for t in tw), nt


def _fold_weights(wkv, bkv, wk_rpe, bk_rpe, wq_rpe, bq_rpe):
    wkv = np.asarray(wkv, np.float32)
    bkv = np.asarray(bkv, np.float32)
    W = np.zeros((128, 128), np.float32)
    W[0:DIM, 0:DH] = wkv[:, :DH]                    # k from x
    W[DIM:DIM + RPE, 0:DH] = np.asarray(wk_rpe, np.float32)
    W[0:DIM, DH:DH + DIM] = wkv[:, DH:]             # v from x
    W[DIM:DIM + RPE, 96:128] = np.asarray(wq_rpe, np.float32)
    W[DIM + RPE:DIM + RPE + DH, 96:128] = np.eye(DH, dtype=np.float32)
    b = np.zeros(128, np.float32)
    b[0:DH] = bkv[:DH] + np.asarray(bk_rpe, np.float32)
    b[DH:DH + DIM] = bkv[DH:]
    b[96:128] = np.asarray(bq_rpe, np.float32)
    W[KROWS - 1, :] = b                             # bias via ones row
    return W.astype(F16), b


def _build(tw, nt, reps=1, ablate=()):
    import concourse.bacc as bacc
    import concourse.tile as tile
    import concourse.bass as bass
    from concourse import mybir

    f16 = mybir.dt.float16
    f32 = mybir.dt.float32
    npc = nt * CTILE

    nc = bacc.Bacc("TRN2", target_bir_lowering=False, debug=False,
                   num_devices=NCORES)
    xq_d = nc.dram_tensor("xq", [XROWS, npc], f16, kind="ExternalInput")
    widx_d = nc.dram_tensor("widx", [CTILE, nt], f32, kind="ExternalInput")
    wcomb_d = nc.dram_tensor("wcomb", [XROWS, 128], f16, kind="ExternalInput")
    iota_d = nc.dram_tensor("iota", [CTILE, CTILE], f16, kind="ExternalInput")
    out_d = nc.dram_tensor("out", [NWIN * WIN, DIM], f32, kind="ExternalOutput")

    with tile.TileContext(nc) as tc:
        with (
            tc.tile_pool(name="const", bufs=1) as constp,
            tc.tile_pool(name="xq", bufs=3) as xqp,
            tc.tile_pool(name="projps", bufs=2, space="PSUM") as projps,
            tc.tile_pool(name="winps", bufs=4, space="PSUM") as winps,
            tc.tile_pool(name="onehot", bufs=6) as onehotp,
            tc.tile_pool(name="qk", bufs=3) as qkp,
            tc.tile_pool(name="compat", bufs=3) as compatp,
            tc.tile_pool(name="feat", bufs=5) as featp,
            tc.tile_pool(name="fin", bufs=2) as finp,
        ):
            w_sb = constp.tile([XROWS, 128], f16)
            nc.sync.dma_start(w_sb[:], wcomb_d.ap())
            iota_sb = constp.tile([CTILE, CTILE], f16)
            nc.sync.dma_start(iota_sb[:], iota_d.ap())
            widx_sb = constp.tile([CTILE, nt], f32)
            nc.sync.dma_start(widx_sb[:], widx_d.ap())

            import contextlib
            rep_loop = tc.For_i(0, reps, 1) if reps > 1 else contextlib.nullcontext()
            rep_loop.__enter__()

            # tile tau -> window
            t2w = []
            for w_i, t_n in enumerate(tw):
                t2w += [w_i] * t_n
            last_of_win = {}
            for tau, w_i in enumerate(t2w):
                last_of_win[w_i] = tau

            xq_sb = None
            pp = None
            win_ps = {}
            pending = []  # (tau, chunk_slot) scatters waiting on feat
            feat_sb = None
            flush_q = []  # delayed (feat, taus) so PE FIFO isn't blocked
            FLUSH_DELAY = 2

            def flush_chunk(feat_t, taus):
                if "noscat" in ablate:
                    return
                # scatter each tile of the finished chunk into its window psum
                for (tau, b) in taus:
                    w_i = t2w[tau]
                    if w_i not in win_ps:
                        win_ps[w_i] = winps.tile([WIN, 68], f32, tag="winps", name="winacc")
                    if "onehot" in ablate:
                        oh = iota_sb  # constant stand-in, wrong results
                    else:
                        oh = onehotp.tile([CTILE, WIN], f16)
                        nc.vector.tensor_scalar(
                            oh[:], iota_sb[:], widx_sb[:, tau:tau + 1], None,
                            mybir.AluOpType.is_equal)
                    first = (tau == tile_off_first[w_i])
                    last = (tau == last_of_win[w_i])
                    fw = feat_t.shape[1] if hasattr(feat_t, 'shape') else 0
                    off = (b * 68) if (b + 1) * 68 <= fw else 0
                    nc.tensor.matmul(
                        win_ps[w_i][:], oh[:], feat_t[:, off:off + 68],
                        start=first, stop=last)
                    if last:
                        _finalize(w_i)

            OB = 7  # windows per output DMA (98 = 14*7)
            ob_state = {"tile": None}

            def _finalize(w_i):
                ps = win_ps.pop(w_i)
                sinv = finp.tile([WIN, H], f32, tag="sinv")
                nc.scalar.activation(sinv[:], ps[:, 64:68],
                                     mybir.ActivationFunctionType.Copy,
                                     bias=1e-16)
                nc.vector.reciprocal(sinv[:], sinv[:])
                slot = w_i % OB
                if slot == 0:
                    ob_state["tile"] = finp.tile([WIN, OB * DIM], f32,
                                                 tag="osb", name="obatch")
                o_sb = ob_state["tile"]
                sb_ap = bass.AP(tensor=sinv[:].tensor, offset=sinv[:].offset,
                                ap=[list(sinv[:].ap[0]), [1, H], [0, 16]])
                nc.vector.tensor_tensor(
                    o_sb[:, slot * DIM:(slot + 1) * DIM], ps[:, 0:64], sb_ap,
                    mybir.AluOpType.mult)
                if slot == OB - 1:
                    w0 = w_i - OB + 1
                    oda = out_d.ap()
                    dst = bass.AP(tensor=oda.tensor,
                                  offset=w0 * WIN * DIM,
                                  ap=[[DIM, WIN], [WIN * DIM, OB], [1, DIM]])
                    nc.sync.dma_start(dst, o_sb[:])

            tile_off_first = {}
            tau = 0
            for w_i, t_n in enumerate(tw):
                tile_off_first[w_i] = tau
                tau += t_n

            for tau in range(nt):
                j = tau % LOAD_TILES
                if j == 0:
                    xq_sb = xqp.tile([XROWS, LOAD_TILES * CTILE], f16)
                    nc.sync.dma_start(
                        xq_sb[:],
                        xq_d.ap()[:, tau * CTILE:(tau + LOAD_TILES) * CTILE])
                b = tau % CHUNK
                if b == 0:
                    pp = projps.tile([128, CHUNK * 128], f32)
                    pending = []
                if "noproj" not in ablate:
                    nc.tensor.matmul(
                        pp[:, b * 128:(b + 1) * 128],
                        xq_sb[:, j * CTILE:(j + 1) * CTILE], w_sb[:],
                        start=True, stop=True)
                pending.append((tau, b))
                if b == CHUNK - 1:
                    # evacuate whole proj chunk (k|v|q per tile) to SBUF fp16
                    ppa = pp[:]
                    if "evac" in ablate:
                        flush_chunk(iota_sb, pending)
                        continue
                    kqv = qkp.tile([128, CHUNK * 128], f16, tag="kqv")
                    nc.scalar.activation(kqv[:], ppa,
                                         mybir.ActivationFunctionType.Copy)
                    if "elem" in ablate:
                        feat_sb = kqv
                        class _FS:  # slice feat from kqv layout
                            pass
                        flush_chunk(kqv, [(t_, b_) for (t_, b_) in pending])
                        continue
                    qk = qkp.tile([128, CHUNK * DH], f16, tag="qk")
                    kqa = kqv[:]
                    q_ap = bass.AP(tensor=kqa.tensor, offset=kqa.offset + 96,
                                   ap=[list(kqa.ap[0]), [128, CHUNK], [1, DH]])
                    k_ap = bass.AP(tensor=kqa.tensor, offset=kqa.offset,
                                   ap=[list(kqa.ap[0]), [128, CHUNK], [1, DH]])
                    nc.vector.tensor_tensor(qk[:], q_ap, k_ap,
                                            mybir.AluOpType.mult)
                    compat = compatp.tile([128, CHUNK * H], f32)
                    qkv_v = qk[:].rearrange("p (t h d) -> p (t h) d", h=H, d=DQK)
                    nc.vector.tensor_reduce(
                        compat[:], qkv_v, mybir.AxisListType.X,
                        mybir.AluOpType.add)
                    feat_sb = featp.tile([128, CHUNK * 68], f16)
                    fa = feat_sb[:]
                    e_ap = bass.AP(tensor=fa.tensor, offset=fa.offset + 64,
                                   ap=[list(fa.ap[0]), [68, CHUNK], [1, H]])
                    nc.scalar.activation(e_ap, compat[:],
                                         mybir.ActivationFunctionType.Exp)
                    wv_ap = bass.AP(tensor=fa.tensor, offset=fa.offset,
                                    ap=[list(fa.ap[0]), [68, CHUNK], [1, DIM]])
                    v_ap = bass.AP(tensor=kqa.tensor, offset=kqa.offset + DH,
                                   ap=[list(kqa.ap[0]), [128, CHUNK], [1, DIM]])
                    eb_ap = bass.AP(tensor=fa.tensor, offset=fa.offset + 64,
                                    ap=[list(fa.ap[0]), [68, CHUNK], [1, H],
                                        [0, 16]])
                    nc.vector.tensor_tensor(wv_ap, v_ap, eb_ap,
                                            mybir.AluOpType.mult)
                    flush_q.append((feat_sb, pending))
                    if len(flush_q) > FLUSH_DELAY:
                        flush_chunk(*flush_q.pop(0))
            for fq in flush_q:
                flush_chunk(*fq)
            flush_q = []
            rep_loop.__exit__(None, None, None)
    nc.compile()
    return nc


def kernel(**inputs):
    from concourse.bass_utils import run_bass_kernel_spmd

    in_maps, tw, nt = _host_prep(**inputs)
    key = (tw, nt)
    if key not in _BUILD_CACHE:
        _BUILD_CACHE[key] = _build(tw, nt)
    nc = _BUILD_CACHE[key]
    res = run_bass_kernel_spmd(nc, in_maps, list(range(NCORES)))
    out = np.concatenate(
        [res.results[c]["out"][:PPC] for c in range(NCORES)], axis=0)
    return out.astype(np.float32)

